# revision 1
# baseline (speedup 1.0000x reference)
"""Trainium2 Bass kernel for nn_EncodingLayer_47261820125416.

ALBERT-style encoder layer integrated with 4 fixed RKF56 steps (24 dyn()
evaluations). B=4 batch elements over 8 cores: each core runs TWO batch
elements (E=2) interleaved; core c computes elements (2c)%4 and (2c+1)%4
(cores 2-7 duplicate cores 0-1). Attention + FFN mix all tokens/dims of one
element and collective latency >> any gain, so no cross-core communication.

Why two elements per core: the ACT engine (the only exp engine) is the
bottleneck; one element's dyn() is ~16.6us of exp plus a ~8us serial tail
(AV tail -> ctx norm -> Wo -> FFN -> k -> shortcut QK -> scores) in which
ACT idles.  Interleaving a second element's exp phase into that tail keeps
ACT saturated.  Per stage the emission order is:
  ph1(A) [scores+exps] ; ph1(B) ; tail(A) ; tail(B)
so each engine's FIFO sees A's exps, then B's exps, while A's tail (PE/DVE
work) executes during B's exp phase and vice versa.

Per-core per-element design (S=512 tokens, d=64, H=8 heads of dh=8, DFF=256):
- State kept transposed on-chip: yT [64, S] fp32, with a ones-row appended so
  every projection bias rides along as an extra contraction row (K=65).
- Scores are computed TRANSPOSED (scoresT[h] = [keys, queries]) with 16-way
  32x32 PE tiling: a padded/duplicated head layout (each 32-partition group
  holds one head's 8 dims) lets one 16-tile pack compute 2 heads x 2
  key-chunks x 512 queries in ~330ns.
- softmax: exp on ACT reading [128, 1024] PSUM spans; no max subtraction
  (scores are bounded ~22, validated numerically); the additive mask is a
  no-op under softmax (it broadcasts over keys) and is ignored.
- The softmax denominator Z comes out of the attn@V matmul for free via a
  ones-column appended to V; AV consumes exp-scores as bf16 FWL stationaries
  ([128k, 128q] blocks) producing ctx in [queries, head-major] layout where
  1/Z is a per-partition scalar.
- FFN/projections in bf16, RK linear combinations via fp32r matmuls against
  precomputed coefficient matrices; y accumulated in fp32 on DVE.
- hp0's Q|K for the next dyn comes from a linearity shortcut
  (QK(partial) + a*QK(k_last)) and is evacuated to SBUF inside the
  producing dyn's tail.
"""

import numpy as np
import ml_dtypes

DEPTH, HEADS, DFF = 64, 8, 256
DH = DEPTH // HEADS
S = 512
B = 4
E = 2  # batch elements per core
N_CORES = 8
T_FINAL, DT = 1.0, 0.25
N_STEPS = int(round(T_FINAL / DT))
SCALE = 1.0 / float(np.sqrt(DH))

# RKF56 (Fehlberg 6-stage) coefficients
A_COEF = [
    [],
    [0.25],
    [3.0 / 32.0, 9.0 / 32.0],
    [1932.0 / 2197.0, -7200.0 / 2197.0, 7296.0 / 2197.0],
    [439.0 / 216.0, -8.0, 3680.0 / 513.0, -845.0 / 4104.0],
    [-8.0 / 27.0, 2.0, -3544.0 / 2565.0, 1859.0 / 4104.0, -11.0 / 40.0],
]
B_COEF = [16.0 / 135.0, 0.0, 6656.0 / 12825.0, 28561.0 / 56430.0, -9.0 / 50.0, 2.0 / 55.0]


def _partial_coef_list():
    """Coefficient pairs for partial_j = y + sum_{i<=j-2} a_ji k'_i (stages 2..5)
    and partial_final = y + b1 k1 + b3 k3 + b4 k4."""
    out = []
    for i in range(2, 6):
        a = A_COEF[i][:-1]  # exclude the last (pending) k
        pairs = []
        for pi in range((len(a) + 1) // 2):
            ce = a[2 * pi]
            co = a[2 * pi + 1] if 2 * pi + 1 < len(a) else 0.0
            pairs.append((pi, ce, co))
        out.append(pairs)
    out.append([(0, B_COEF[0], 0.0), (1, B_COEF[2], B_COEF[3])])
    return out


def _shortcut_coefs():
    """Per-stage (lo, hi) coefficients of the kp-pair used by the hp0-QK
    shortcut matmul, and which pair it reads. Stage j (1..5) uses k'_{j-1};
    the final/stage-0 shortcut uses (b5 k'5 + b6 k'6) on pair 2."""
    out = []
    for j in range(1, 6):
        a = A_COEF[j][j - 1]
        pair = (j - 1) // 2
        lo, hi = (a, 0.0) if (j - 1) % 2 == 0 else (0.0, a)
        out.append((pair, lo, hi))
    out.append((2, B_COEF[4], B_COEF[5]))
    return out


def _coef_list():
    """[(stage, [(pair_idx, c_even, c_odd), ...])] for stages 1..5 and final."""
    out = []
    for i in range(1, 6):
        a = A_COEF[i]
        pairs = []
        for pi in range((len(a) + 1) // 2):
            ce = a[2 * pi]
            co = a[2 * pi + 1] if 2 * pi + 1 < len(a) else 0.0
            pairs.append((pi, ce, co))
        out.append(pairs)
    fb = []
    for pi in range(3):
        fb.append((pi, B_COEF[2 * pi], B_COEF[2 * pi + 1]))
    out.append(fb)
    return out


def prepare_aux(inputs):
    """Host-side numpy preparation of padded/duplicated weight layouts."""
    f32 = np.float32
    bf16 = ml_dtypes.bfloat16
    Wq, bq = np.asarray(inputs["Wq"], f32), np.asarray(inputs["bq"], f32)
    Wk, bk = np.asarray(inputs["Wk"], f32), np.asarray(inputs["bk"], f32)
    Wv, bv = np.asarray(inputs["Wv"], f32), np.asarray(inputs["bv"], f32)
    Wo, bo = np.asarray(inputs["Wo"], f32), np.asarray(inputs["bo"], f32)
    W1, b1 = np.asarray(inputs["W1"], f32), np.asarray(inputs["b1"], f32)
    W2, b2 = np.asarray(inputs["W2"], f32), np.asarray(inputs["b2"], f32)

    # wqk[w, hp, 65, 128]: for head pair hp = (2hp, 2hp+1), column layout
    # [headA pad32 | headB pad32 | headA pad32 | headB pad32], row 64 = bias.
    wqk = np.zeros((2, 4, DEPTH + 1, 128), f32)
    for wi, (W, bias) in enumerate(((Wq, bq), (Wk, bk))):
        for hp in range(4):
            for r in range(4):
                h = 2 * hp + (r % 2)
                wqk[wi, hp, :DEPTH, 32 * r:32 * r + DH] = W[:, h * DH:(h + 1) * DH]
                wqk[wi, hp, DEPTH, 32 * r:32 * r + DH] = bias[h * DH:(h + 1) * DH]

    wv_aug = np.concatenate([Wv, bv[None, :]], 0).astype(f32)          # [65, 64]
    # bo is folded linearly: z' = y + att_no_bo, relu(z'W1 + (b1 + bo@W1)),
    # and k-evac adds (b2 + bo) per-partition. The proj then has no bias row.
    wo_aug = np.concatenate([Wo, 0 * bo[None, :]], 0).astype(bf16)     # [65, 64]
    w1_aug = np.concatenate([W1, (b1 + bo @ W1)[None, :]], 0).astype(bf16)  # [65, 256]
    w2_ch = W2.reshape(2, 128, DFF // 256 * DEPTH).astype(bf16)        # [2, 128, 64]

    coefs = []
    eye = np.eye(DEPTH, dtype=f32)
    for pairs in _coef_list():
        for (pi, ce, co) in pairs:
            m = np.zeros((128, DEPTH), f32)
            m[:DEPTH] = ce * eye
            m[DEPTH:] = co * eye
            coefs.append(m)
    coefs = np.stack(coefs)                                            # [12, 128, 64]

    pcoefs = []
    eye = np.eye(DEPTH, dtype=f32)
    for pairs in _partial_coef_list():
        for (pi, ce, co) in pairs:
            m = np.zeros((128, DEPTH), f32)
            m[:DEPTH] = ce * eye
            m[DEPTH:] = co * eye
            pcoefs.append(m)
    pcoefs = np.stack(pcoefs)                                          # [8, 128, 64]

    # hp0 shortcut stationaries: [6 stages, 2 (q/k), 128, 128]
    wqk0s = np.zeros((6, 2, 128, 128), f32)
    for si, (pair, lo, hi) in enumerate(_shortcut_coefs()):
        for wi in range(2):
            wqk0s[si, wi, :DEPTH, :] = lo * wqk[wi, 0, :DEPTH, :]
            wqk0s[si, wi, DEPTH:, :] = hi * wqk[wi, 0, :DEPTH, :]

    ident = np.eye(128, dtype=f32)
    identb = np.eye(128, dtype=bf16)

    return {
        "wqk": wqk,
        "wv_aug": wv_aug,
        "wo_aug": wo_aug,
        "w1_aug": w1_aug,
        "w2_ch": np.ascontiguousarray(w2_ch),
        "b2_col": np.ascontiguousarray((b2 + bo).reshape(DEPTH, 1)),
        "coefs": coefs,
        "pcoefs": pcoefs,
        "wqk0s": wqk0s,
        "ident": ident,
        "identb": identb,
    }


def build_module(n_steps=N_STEPS, score_mult=1, ablate="none"):
    import concourse.bass as bass  # noqa: F401
    import concourse.mybir as mybir
    import concourse.tile as tile
    from concourse import bacc

    f32 = mybir.dt.float32
    f32r = mybir.dt.float32r
    bf = mybir.dt.bfloat16

    nc = bacc.Bacc("TRN2", target_bir_lowering=False, debug=False, enable_asserts=False)

    # --- DRAM I/O ---
    x_d = nc.dram_tensor("x", [E, S, DEPTH], f32, kind="ExternalInput").ap()
    wqk_d = nc.dram_tensor("wqk", [2, 4, DEPTH + 1, 128], f32r, kind="ExternalInput").ap()
    wv_d = nc.dram_tensor("wv_aug", [DEPTH + 1, DEPTH], f32r, kind="ExternalInput").ap()
    wo_d = nc.dram_tensor("wo_aug", [DEPTH + 1, DEPTH], bf, kind="ExternalInput").ap()
    w1_d = nc.dram_tensor("w1_aug", [DEPTH + 1, DFF], bf, kind="ExternalInput").ap()
    w2_d = nc.dram_tensor("w2_ch", [2, 128, DEPTH], bf, kind="ExternalInput").ap()
    b2_d = nc.dram_tensor("b2_col", [DEPTH, 1], f32, kind="ExternalInput").ap()
    coef_d = nc.dram_tensor("coefs", [12, 128, DEPTH], f32r, kind="ExternalInput").ap()
    pcoef_d = nc.dram_tensor("pcoefs", [8, 128, DEPTH], f32r, kind="ExternalInput").ap()
    wqk0s_d = nc.dram_tensor("wqk0s", [6, 2, 128, 128], f32r, kind="ExternalInput").ap()
    id_d = nc.dram_tensor("ident", [128, 128], f32, kind="ExternalInput").ap()
    idb_d = nc.dram_tensor("identb", [128, 128], bf, kind="ExternalInput").ap()
    y_d = nc.dram_tensor("y_out", [E, S, DEPTH], f32, kind="ExternalOutput").ap()

    coef_stage_index = []
    idx = 0
    for pairs in _coef_list():
        entry = []
        for (pi, _, _) in pairs:
            entry.append((pi, idx))
            idx += 1
        coef_stage_index.append(entry)
    pcoef_stage_index = []
    idx = 0
    for pairs in _partial_coef_list():
        entry = []
        for (pi, _, _) in pairs:
            entry.append((pi, idx))
            idx += 1
        pcoef_stage_index.append(entry)

    with tile.TileContext(nc) as tc:
        with (
            tc.tile_pool(name="persist", bufs=1) as pp,
            tc.tile_pool(name="psc", bufs=2, space="PSUM") as psc,
            tc.tile_pool(name="pctx", bufs=2, space="PSUM") as pctx,
            tc.tile_pool(name="psm", bufs=2, space="PSUM") as psm,
        ):
            # ---- persistent SBUF (per-element state has a leading E dim) ----
            ident = pp.tile([128, 128], f32)
            identb = pp.tile([128, 128], bf)
            wqk_sb = pp.tile([DEPTH + 1, 8, 128], f32r)     # [p, (w hp), m]
            wv_sb = pp.tile([DEPTH + 1, DEPTH], f32r)
            wo_sb = pp.tile([DEPTH + 1, DEPTH], bf)
            w1_sb = pp.tile([DEPTH + 1, DFF], bf)
            w2_sb = pp.tile([128, 2, DEPTH], bf)
            b2_sb = pp.tile([DEPTH, 1], f32)
            coef_sb = pp.tile([128, 12, DEPTH], f32r)
            pcoef_sb = pp.tile([128, 8, DEPTH], f32r)
            wqk0s_sb = pp.tile([128, 6, 2, 128], f32r)
            partial_sb = pp.tile([DEPTH + 1, E, 2, S], f32r)
            yT = pp.tile([DEPTH + 1, E, S], f32)
            ypT = pp.tile([DEPTH + 1, E, 2, S], f32r)
            kp_sb = pp.tile([128, E, 3, S], f32r)
            qkt_sb = pp.tile([128, E, 4, 2, S], bf)  # [:, e, hp, 0]=Q, 1=K
            v_sb = pp.tile([128, E, 4, 72], bf)
            expT = pp.tile([128, E, HEADS, 2, 1024], bf)
            ctxn = pp.tile([128, E, 4, DEPTH], bf)
            ctxT_sb = pp.tile([DEPTH + 1, E, S], bf)
            zT_sb = pp.tile([DEPTH + 1, E, S], bf)
            hT_sb = pp.tile([128, E, 2, S], bf)
            recipZ = pp.tile([128, E, 4, HEADS], f32)
            xn_sb = pp.tile([128, E, 4, DEPTH], f32)
            yout_sb = pp.tile([128, E, 4, DEPTH], f32)

            # ---- prologue: weight DMAs + presets ----
            nc.sync.dma_start(ident, id_d)
            nc.sync.dma_start(identb, idb_d)
            nc.sync.dma_start(wqk_sb, wqk_d.rearrange("w h p m -> p (w h) m"))
            nc.sync.dma_start(wv_sb, wv_d)
            nc.sync.dma_start(wo_sb, wo_d)
            nc.sync.dma_start(w1_sb, w1_d)
            nc.sync.dma_start(w2_sb, w2_d.rearrange("c p m -> p c m"))
            nc.sync.dma_start(b2_sb, b2_d)
            nc.sync.dma_start(coef_sb, coef_d.rearrange("s p m -> p s m"))
            nc.sync.dma_start(pcoef_sb, pcoef_d.rearrange("s p m -> p s m"))
            nc.sync.dma_start(wqk0s_sb, wqk0s_d.rearrange("s w p m -> p s w m"))
            nc.sync.dma_start(xn_sb, x_d.rearrange("e (c p) d -> p e c d", p=128))

            nc.vector.memset(kp_sb.bitcast(f32), 0.0)
            nc.vector.memset(yT[DEPTH:DEPTH + 1, :, :], 1.0)
            nc.vector.memset(ypT.bitcast(f32)[DEPTH:DEPTH + 1, :, :, :], 1.0)
            nc.vector.memset(partial_sb.bitcast(f32)[DEPTH:DEPTH + 1, :, :, :], 1.0)
            nc.vector.memset(ctxT_sb[DEPTH:DEPTH + 1, :, :], 1.0)
            nc.vector.memset(zT_sb[DEPTH:DEPTH + 1, :, :], 1.0)
            v4 = v_sb.rearrange("p e c (h n) -> p e c h n", h=HEADS)
            nc.vector.memset(v_sb, 1.0)  # ones column at [..., 8] survives V writes
            if ablate == "noexp":
                nc.vector.memset(expT, 1.0)

            tc.strict_bb_all_engine_barrier()

            # transpose input x -> yT
            for e in range(E):
                yt0 = psm.tile([DEPTH, S], f32, tag="sm")
                for qc in range(4):
                    nc.tensor.transpose(
                        yt0[:, 128 * qc:128 * qc + 128], xn_sb[:, e, qc, :], ident
                    )
                nc.vector.tensor_copy(yT[:DEPTH, e, :], yt0)

            def emit_qk(e, yp_ap, hp):
                t = psc.tile([128, 1024], f32, tag="sc")
                nc.tensor.matmul(t[:, 0:512], wqk_sb[:, hp, :], yp_ap)
                nc.tensor.matmul(t[:, 512:1024], wqk_sb[:, 4 + hp, :], yp_ap)
                nc.vector.tensor_copy(
                    qkt_sb[:, e, hp, :, :],
                    t.rearrange("p (w n) -> p w n", w=2),
                )

            def emit_v(e, yp_ap):
                tv = psm.tile([128, S], f32, tag="sm")
                for c in range(4):
                    nc.tensor.matmul(
                        tv[:, DEPTH * c:DEPTH * (c + 1)],
                        yp_ap[:, 128 * c:128 * c + 128], wv_sb,
                    )
                nc.vector.tensor_copy(
                    v4[:, e, :, :, :DH],
                    tv[:, :4 * DEPTH].rearrange("p (c h n) -> p c h n", c=4, h=HEADS),
                )

            def emit_pack(e, hp, kp):
                tiles = []
                for hsel in range(2):
                    t = psc.tile([128, 1024], f32, tag="sc")
                    tiles.append(t)
                    for rep in range(score_mult):
                        for rr in range(2):
                            r = hsel + 2 * rr  # row group; rr selects key chunk
                            kc = 2 * kp + rr
                            for c in range(4):
                                nc.tensor.matmul(
                                    t[32 * c:32 * c + 32, 512 * rr:512 * rr + 512],
                                    qkt_sb[:, e, hp, 1, :][
                                        32 * r:32 * r + 32,
                                        128 * kc + 32 * c:128 * kc + 32 * c + 32,
                                    ],
                                    qkt_sb[:, e, hp, 0, :][32 * r:32 * r + 32, :],
                                    tile_position=(32 * r, 32 * c),
                                )
                import concourse.mybir as mybir
                if ablate != "noexp":
                    for hsel in range(2):
                        h = 2 * hp + hsel
                        nc.scalar.activation(
                            expT[:, e, h, kp, :], tiles[hsel][:, :],
                            mybir.ActivationFunctionType.Exp, scale=SCALE,
                        )

            def emit_ph1(e, yp_ap, qk0_ready):
                """Scores + exps for all 8 packs; QKV projections woven in."""
                if not qk0_ready:
                    emit_qk(e, yp_ap, 0)
                emit_pack(e, 0, 0)
                emit_qk(e, yp_ap, 1)
                emit_pack(e, 0, 1)
                emit_qk(e, yp_ap, 2)
                emit_v(e, yp_ap)
                emit_pack(e, 1, 0)
                emit_qk(e, yp_ap, 3)
                emit_pack(e, 1, 1)
                emit_pack(e, 2, 0)
                emit_pack(e, 2, 1)
                emit_pack(e, 3, 0)
                emit_pack(e, 3, 1)

            def emit_tail(e, yp_ap, k_slot, next_sc):
                """AV + ctx norm + Wo + FFN + k evac + next-dyn hp0 shortcut."""
                import concourse.mybir as mybir
                ctx_ps = pctx.tile([128, 288], f32, tag="ctx")
                attk_ps = pctx.tile([DEPTH, S], f32, tag="ctx")
                ctxT_ps = psm.tile([DEPTH, S], bf, tag="sm")
                c3 = ctx_ps.rearrange("p (q h n) -> p q h n", q=4, h=HEADS)

                def emit_av(h):
                    for qc in range(4):
                        for kc in range(4):
                            nc.tensor.matmul(
                                ctx_ps[:, 72 * qc + 9 * h:72 * qc + 9 * h + 9],
                                expT[:, e, h, kc // 2,
                                     (kc % 2) * 512 + 128 * qc:(kc % 2) * 512 + 128 * qc + 128],
                                v_sb[:, e, kc, 9 * h:9 * h + 9],
                                start=(kc == 0), stop=(kc == 3),
                            )

                def emit_ctx_half(g):
                    hs = slice(4 * g, 4 * g + 4)
                    for qc in range(4):
                        nc.vector.reciprocal(recipZ[:, e, qc, hs], c3[:, qc, hs, DH])
                        nc.vector.tensor_tensor(
                            ctxn[:, e, qc, 32 * g:32 * g + 32].rearrange(
                                "p (h n) -> p h n", h=4),
                            c3[:, qc, hs, :DH],
                            recipZ[:, e, qc, hs, None].to_broadcast((128, 4, DH)),
                            mybir.AluOpType.mult,
                        )
                        nc.tensor.transpose(
                            ctxT_ps[32 * g:32 * g + 32, 128 * qc:128 * qc + 128],
                            ctxn[:, e, qc, 32 * g:32 * g + 32], identb,
                            tile_position=(0, 32 * g),
                        )
                    if g == 0:
                        nc.vector.tensor_copy(ctxT_sb[0:32, e, :], ctxT_ps[0:32, :])
                        nc.tensor.matmul(attk_ps, wo_sb[0:32, :], ctxT_sb[0:32, e, :],
                                         start=True, stop=False, skip_group_check=True)
                    else:
                        nc.vector.tensor_copy(ctxT_sb[32:DEPTH, e, :],
                                              ctxT_ps[32:DEPTH, :])
                        nc.tensor.matmul(attk_ps, wo_sb[32:DEPTH, :],
                                         ctxT_sb[32:DEPTH, e, :],
                                         start=False, stop=True, skip_group_check=True)

                emit_av(0)
                emit_av(1)
                emit_av(2)
                emit_av(3)
                emit_ctx_half(0)
                emit_av(4)
                emit_av(5)
                emit_av(6)
                emit_av(7)

                # next dyn's partial (off the critical path)
                mm1_rhs = None
                if next_sc is not None:
                    sidx, mm1_rhs, pair_idx, pemit, pslot = next_sc
                    if pemit is not None:
                        d_p = psm.tile([DEPTH, S], f32, tag="sm")
                        for j, (pi, ci) in enumerate(pemit):
                            nc.tensor.matmul(
                                d_p, pcoef_sb[:, ci, :], kp_sb[:, e, pi, :],
                                start=(j == 0), stop=(j == len(pemit) - 1),
                            )
                        nc.vector.tensor_add(
                            partial_sb[:DEPTH, e, pslot, :], yT[:DEPTH, e, :], d_p)
                        mm1_rhs = partial_sb[:, e, pslot, :]

                emit_ctx_half(1)

                yp_f = yp_ap.bitcast(f32)
                ksl = kp_sb[64 * (k_slot % 2):64 * (k_slot % 2) + 64, e,
                            k_slot // 2, :]
                if ablate != "noffn":
                    nc.vector.tensor_add(zT_sb[:DEPTH, e, :], yp_f[:DEPTH, :],
                                         attk_ps)
                    for ch in range(2):
                        th = psm.tile([128, S], f32, tag="sm")
                        nc.tensor.matmul(th, w1_sb[:, 128 * ch:128 * ch + 128],
                                         zT_sb[:, e, :])
                        nc.vector.tensor_scalar_max(hT_sb[:, e, ch, :], th, 0.0)
                    for ch in range(2):
                        nc.tensor.matmul(
                            attk_ps, w2_sb[:, ch, :], hT_sb[:, e, ch, :],
                            start=False, stop=(ch == 1), skip_group_check=True,
                        )
                # shortcut Q|K for the next dyn's hp0: two 1-bank psm
                # tiles (NOT the psc rotation, where the allocation would
                # WAR-wait on the other element's last exp).  mm1 (partial
                # part) needs no k and overlaps the k evac below; mm2 adds
                # a*QK(k_last); both are evacuated to SBUF here in the tail.
                qk_next = None
                if next_sc is not None:
                    qkQ = psm.tile([128, S], f32, tag="sm")
                    qkK = psm.tile([128, S], f32, tag="sm")
                    qk_next = (qkQ, qkK)
                    nc.tensor.matmul(qkQ, wqk_sb[:, 0, :], mm1_rhs,
                                     start=True, stop=False, skip_group_check=True)
                    nc.tensor.matmul(qkK, wqk_sb[:, 4, :], mm1_rhs,
                                     start=True, stop=False, skip_group_check=True)
                nc.vector.tensor_scalar(
                    ksl, attk_ps, b2_sb, DT,
                    mybir.AluOpType.add, mybir.AluOpType.mult,
                )
                if qk_next is not None:
                    sidx = next_sc[0]
                    pair_idx = next_sc[2]
                    nc.tensor.matmul(qkQ, wqk0s_sb[:, sidx, 0, :],
                                     kp_sb[:, e, pair_idx, :],
                                     start=False, stop=True, skip_group_check=True)
                    nc.tensor.matmul(qkK, wqk0s_sb[:, sidx, 1, :],
                                     kp_sb[:, e, pair_idx, :],
                                     start=False, stop=True, skip_group_check=True)
                    nc.vector.tensor_copy(qkt_sb[:, e, 0, 0, :], qkQ)
                    nc.vector.tensor_copy(qkt_sb[:, e, 0, 1, :], qkK)
                    return True
                return None

            def emit_delta(e, stage_pairs):
                d_ps = psm.tile([DEPTH, S], f32, tag="sm")
                n = len(stage_pairs)
                for j, (pi, ci) in enumerate(stage_pairs):
                    nc.tensor.matmul(
                        d_ps, coef_sb[:, ci, :],
                        kp_sb[:, e, pi, :],
                        start=(j == 0), stop=(j == n - 1),
                    )
                return d_ps

            use_sc = (ablate == "none") or (ablate == "noffn")
            qk0_ready = [None] * E
            yp_aps = [None] * E

            def emit_head(e, step, st):
                """Stage-head for element e: build yp, project Q/K (hp1-3),
                V, all score packs + exps."""
                if st == 0:
                    yp_ap = ypT[:, e, 0, :]
                    nc.vector.tensor_copy(yp_ap[:DEPTH, :], yT[:DEPTH, e, :])
                    if use_sc and qk0_ready[e] is None:
                        # very first dyn: direct hp0 QK from y
                        qk0 = psc.tile([128, 1024], f32, tag="sc")
                        nc.tensor.matmul(qk0[:, 0:512], wqk_sb[:, 0, :], yp_ap)
                        nc.tensor.matmul(qk0[:, 512:1024], wqk_sb[:, 4, :], yp_ap)
                        nc.vector.tensor_copy(
                            qkt_sb[:, e, 0, :, :],
                            qk0.rearrange("p (w n) -> p w n", w=2))
                        qk0_ready[e] = True
                else:
                    d_ps = emit_delta(e, coef_stage_index[st - 1])
                    yp_ap = ypT[:, e, st % 2, :]
                    nc.vector.tensor_add(yp_ap[:DEPTH, :], yT[:DEPTH, e, :], d_ps)
                yp_aps[e] = yp_ap
                emit_ph1(e, yp_ap, use_sc and qk0_ready[e])

            def next_sc_for(step, st):
                if not use_sc:
                    return None
                last = (step == n_steps - 1) and (st == 5)
                if last:
                    return None
                if st < 5:
                    j = st + 1  # next stage 1..5, shortcut idx j-1
                    pemit = (None if j < 2 else pcoef_stage_index[j - 2])
                    return (j - 1, None if j > 1 else None, (j - 1) // 2,
                            pemit, j % 2)
                # next is stage 0 of the next step: partial_f route
                return (5, None, 2, pcoef_stage_index[4], 0)

            # Flat (step, stage) schedule.  Per slot and element:
            #   tail(e, st) ; [yT update if st==5] ; head(e, next slot)
            # so ACT's FIFO alternates A-exps / B-exps with each element's
            # tail overlapped by the other element's exp train.
            seq = [(step, st) for step in range(n_steps) for st in range(6)]
            for e in range(E):
                emit_head(e, *seq[0])
            for i, (step, st) in enumerate(seq):
                for e in range(E):
                    sc = next_sc_for(step, st)
                    if sc is not None and st == 0:
                        # stage-1 shortcut mm1 reads yp directly
                        sc = (sc[0], yp_aps[e], sc[2], sc[3], sc[4])
                    r = emit_tail(e, yp_aps[e], st, sc)
                    if use_sc:
                        qk0_ready[e] = r
                    if st == 5:
                        d_ps = emit_delta(e, coef_stage_index[5])
                        nc.vector.tensor_add(yT[:DEPTH, e, :], yT[:DEPTH, e, :],
                                             d_ps)
                    if i + 1 < len(seq):
                        emit_head(e, *seq[i + 1])

            # ---- epilogue: transpose yT back and store ----
            for e in range(E):
                yo = pctx.tile([128, 288], f32, tag="ctx")
                for qc in range(4):
                    nc.tensor.transpose(
                        yo[:, DEPTH * qc:DEPTH * (qc + 1)],
                        yT[:DEPTH, e, 128 * qc:128 * qc + 128],
                        ident[:DEPTH, :DEPTH],
                    )
                nc.vector.tensor_copy(
                    yout_sb[:, e, :, :], yo[:, :4 * DEPTH].rearrange(
                        "p (c d) -> p c d", c=4)
                )
            nc.sync.dma_start(y_d.rearrange("e (c p) d -> p e c d", p=128), yout_sb)

    nc.compile()
    return nc


def _run(inputs, **spmd_kwargs):
    x = np.asarray(inputs["x"], np.float32)
    aux = prepare_aux(inputs)

    nc = build_module()

    in_maps = []
    for c in range(N_CORES):
        xe = np.stack([x[(2 * c) % B], x[(2 * c + 1) % B]])
        m = {"x": np.ascontiguousarray(xe)}
        m.update(aux)
        in_maps.append(m)

    from concourse.bass_utils import run_bass_kernel_spmd

    res = run_bass_kernel_spmd(nc, in_maps, core_ids=list(range(N_CORES)), **spmd_kwargs)
    out = np.stack([res.results[k // E]["y_out"][k % E] for k in range(B)])
    return out.astype(np.float32), res


def kernel(**inputs):
    return _run(inputs)[0]



# revision 5
# speedup vs baseline: 1.7200x; 1.7200x over previous
"""Trainium2 Bass kernel for nn_EncodingLayer_47261820125416.

ALBERT-style encoder layer integrated with 4 fixed RKF56 steps (24 dyn()
evaluations).  B=4 batch elements over 8 cores in 4 core PAIRS: pair g
computes element g with the 8 attention heads SPLIT across the pair (role
r = core%2 owns heads 4r..4r+3).  The exp() train on ACT -- the bottleneck
engine -- halves versus a single-core element.  Per dyn the pair exchanges
its normalized ctx half ([32,512] bf16, 32KB) via a DRAM-bounce AllGather
(replica groups [[0,1],[2,3],[4,5],[6,7]]); everything after the gathered
ctx (Wo, FFN, k, RK bookkeeping) is replicated on both cores, bitwise
identical, so no further communication is needed.  All role divergence is
carried in per-core input data (wqk / wv / wqk0s slices) -- a single SPMD
module serves both roles.

E=1 (one element per core) beats interleaving two elements per core here:
the serial tail + exchange chain exceeds the halved exp train, so a second
element's hiding is outweighed by paying the chain twice per slot.

Per-core per-element design (S=512 tokens, d=64, H=8 heads of dh=8, DFF=256):
- State kept transposed on-chip: yT [64, S] fp32, with a ones-row appended so
  every projection bias rides along as an extra contraction row (K=65).
- Scores are computed TRANSPOSED (scoresT[h] = [keys, queries]) with 16-way
  32x32 PE tiling: a padded/duplicated head layout (each 32-partition group
  holds one head's 8 dims) lets one 16-tile pack compute 2 heads x 2
  key-chunks x 512 queries in ~330ns.
- softmax: exp on ACT reading [128, 1024] PSUM spans; no max subtraction
  (scores are bounded ~22, validated numerically); the additive mask is a
  no-op under softmax (it broadcasts over keys) and is ignored.
- The softmax denominator Z comes out of the attn@V matmul for free via a
  ones-column appended to V; AV consumes exp-scores as bf16 FWL stationaries
  ([128k, 128q] blocks) producing ctx in [queries, head-major] layout where
  1/Z is a per-partition scalar.
- FFN/projections in bf16, RK linear combinations via fp32r matmuls against
  precomputed coefficient matrices; y accumulated in fp32 on DVE.
- hp0's Q|K for the next dyn comes from a linearity shortcut
  (QK(partial) + a*QK(k_last)) and is evacuated to SBUF inside the
  producing dyn's tail.

Chain-shortening on top of the plain head-split:
- The exchange is pipelined in TWO 16-row chunks: chunk 0 (local heads 0,1)
  is normalized, transposed, and AllGathered mid-ph1, hiding its round trip
  under the second head-pair's exp train; only chunk 1's exchange sits on
  the tail.  Gathered row order is [0,1,4,5,2,3,6,7] (group-rank order);
  Wo / Wo@W1 rows are permuted to match.
- Wo is folded into the FFN first layer: th = W1 @ yp_bf16 (issued before
  the exchange completes) + (Wo@W1) @ ctx (after), removing the serial
  z = y + att DVE add; att itself is still formed for the k evacuation.
"""

import numpy as np
import ml_dtypes

DEPTH, HEADS, DFF = 64, 8, 256
DH = DEPTH // HEADS
S = 512
B = 4
E = 1  # batch elements per core (one per pair; 4 pairs cover B=4)
N_CORES = 8
SPLIT = 2  # cores per element (head-split)
HPL = 4 // SPLIT  # local head pairs
HL = HEADS // SPLIT  # local heads
T_FINAL, DT = 1.0, 0.25
N_STEPS = int(round(T_FINAL / DT))
SCALE = 1.0 / float(np.sqrt(DH))
CC_GROUPS = [[0, 1], [2, 3], [4, 5], [6, 7]]

# RKF56 (Fehlberg 6-stage) coefficients
A_COEF = [
    [],
    [0.25],
    [3.0 / 32.0, 9.0 / 32.0],
    [1932.0 / 2197.0, -7200.0 / 2197.0, 7296.0 / 2197.0],
    [439.0 / 216.0, -8.0, 3680.0 / 513.0, -845.0 / 4104.0],
    [-8.0 / 27.0, 2.0, -3544.0 / 2565.0, 1859.0 / 4104.0, -11.0 / 40.0],
]
B_COEF = [16.0 / 135.0, 0.0, 6656.0 / 12825.0, 28561.0 / 56430.0, -9.0 / 50.0, 2.0 / 55.0]


def _partial_coef_list():
    """Coefficient pairs for partial_j = y + sum_{i<=j-2} a_ji k'_i (stages 2..5)
    and partial_final = y + b1 k1 + b3 k3 + b4 k4."""
    out = []
    for i in range(2, 6):
        a = A_COEF[i][:-1]  # exclude the last (pending) k
        pairs = []
        for pi in range((len(a) + 1) // 2):
            ce = a[2 * pi]
            co = a[2 * pi + 1] if 2 * pi + 1 < len(a) else 0.0
            pairs.append((pi, ce, co))
        out.append(pairs)
    out.append([(0, B_COEF[0], 0.0), (1, B_COEF[2], B_COEF[3])])
    return out


def _shortcut_coefs():
    """Per-stage (lo, hi) coefficients of the kp-pair used by the hp0-QK
    shortcut matmul, and which pair it reads. Stage j (1..5) uses k'_{j-1};
    the final/stage-0 shortcut uses (b5 k'5 + b6 k'6) on pair 2."""
    out = []
    for j in range(1, 6):
        a = A_COEF[j][j - 1]
        pair = (j - 1) // 2
        lo, hi = (a, 0.0) if (j - 1) % 2 == 0 else (0.0, a)
        out.append((pair, lo, hi))
    out.append((2, B_COEF[4], B_COEF[5]))
    return out


def _coef_list():
    """[(stage, [(pair_idx, c_even, c_odd), ...])] for stages 1..5 and final."""
    out = []
    for i in range(1, 6):
        a = A_COEF[i]
        pairs = []
        for pi in range((len(a) + 1) // 2):
            ce = a[2 * pi]
            co = a[2 * pi + 1] if 2 * pi + 1 < len(a) else 0.0
            pairs.append((pi, ce, co))
        out.append(pairs)
    fb = []
    for pi in range(3):
        fb.append((pi, B_COEF[2 * pi], B_COEF[2 * pi + 1]))
    out.append(fb)
    return out


def prepare_aux(inputs, role=0):
    """Host-side numpy preparation of padded/duplicated weight layouts.

    `role` selects which half of the heads this core owns (head-split over
    a core pair): role 0 -> heads 0-3 (head pairs 0,1), role 1 -> heads 4-7
    (head pairs 2,3).  All role divergence is data-driven so a single SPMD
    module serves both roles.
    """
    f32 = np.float32
    bf16 = ml_dtypes.bfloat16
    Wq, bq = np.asarray(inputs["Wq"], f32), np.asarray(inputs["bq"], f32)
    Wk, bk = np.asarray(inputs["Wk"], f32), np.asarray(inputs["bk"], f32)
    Wv, bv = np.asarray(inputs["Wv"], f32), np.asarray(inputs["bv"], f32)
    Wo, bo = np.asarray(inputs["Wo"], f32), np.asarray(inputs["bo"], f32)
    W1, b1 = np.asarray(inputs["W1"], f32), np.asarray(inputs["b1"], f32)
    W2, b2 = np.asarray(inputs["W2"], f32), np.asarray(inputs["b2"], f32)

    # wqk[w, hp, 65, 128]: for head pair hp = (2hp, 2hp+1), column layout
    # [headA pad32 | headB pad32 | headA pad32 | headB pad32], row 64 = bias.
    wqk_full = np.zeros((2, 4, DEPTH + 1, 128), f32)
    for wi, (W, bias) in enumerate(((Wq, bq), (Wk, bk))):
        for hp in range(4):
            for r in range(4):
                h = 2 * hp + (r % 2)
                wqk_full[wi, hp, :DEPTH, 32 * r:32 * r + DH] = W[:, h * DH:(h + 1) * DH]
                wqk_full[wi, hp, DEPTH, 32 * r:32 * r + DH] = bias[h * DH:(h + 1) * DH]

    my_hps = list(range(role * HPL, (role + 1) * HPL))
    my_lo, my_hi = role * HL * DH, (role + 1) * HL * DH
    wqk = np.ascontiguousarray(wqk_full[:, my_hps])                    # [2, HPL, 65, 128]

    wv_full = np.concatenate([Wv, bv[None, :]], 0).astype(f32)         # [65, 64]
    wv_aug = np.ascontiguousarray(wv_full[:, my_lo:my_hi])             # [65, 32]
    # bo is folded linearly: z' = y + att_no_bo, relu(z'W1 + (b1 + bo@W1)),
    # and k-evac adds (b2 + bo) per-partition. The proj then has no bias row.
    # Canonical gathered ctx row order: chunk g2 holds local heads
    # (2g2, 2g2+1) of the even core then the odd core, i.e. global heads
    # [0,1,4,5 | 2,3,6,7].  Wo (and Wo@W1) rows are permuted to match.
    perm = [0, 1, 4, 5, 2, 3, 6, 7]
    prow = np.concatenate([np.arange(8 * h, 8 * h + 8) for h in perm])
    wo_aug = np.concatenate([Wo[prow], 0 * bo[None, :]], 0).astype(bf16)  # [65, 64]
    wow1 = np.ascontiguousarray((Wo @ W1)[prow]).astype(bf16)          # [64, 256]
    w1_aug = np.concatenate([W1, (b1 + bo @ W1)[None, :]], 0).astype(bf16)  # [65, 256]
    w2_ch = W2.reshape(2, 128, DFF // 256 * DEPTH).astype(bf16)        # [2, 128, 64]

    coefs = []
    eye = np.eye(DEPTH, dtype=f32)
    for pairs in _coef_list():
        for (pi, ce, co) in pairs:
            m = np.zeros((128, DEPTH), f32)
            m[:DEPTH] = ce * eye
            m[DEPTH:] = co * eye
            coefs.append(m)
    coefs = np.stack(coefs)                                            # [12, 128, 64]

    pcoefs = []
    eye = np.eye(DEPTH, dtype=f32)
    for pairs in _partial_coef_list():
        for (pi, ce, co) in pairs:
            m = np.zeros((128, DEPTH), f32)
            m[:DEPTH] = ce * eye
            m[DEPTH:] = co * eye
            pcoefs.append(m)
    pcoefs = np.stack(pcoefs)                                          # [8, 128, 64]

    # first-local-hp shortcut stationaries: [6 stages, 2 (q/k), 128, 128]
    wqk0s = np.zeros((6, 2, 128, 128), f32)
    for si, (pair, lo, hi) in enumerate(_shortcut_coefs()):
        for wi in range(2):
            wqk0s[si, wi, :DEPTH, :] = lo * wqk[wi, 0, :DEPTH, :]
            wqk0s[si, wi, DEPTH:, :] = hi * wqk[wi, 0, :DEPTH, :]
    # (wqk[wi, 0] is already this role's first local head pair)

    ident = np.eye(128, dtype=f32)
    identb = np.eye(128, dtype=bf16)

    return {
        "wqk": wqk,
        "wv_aug": wv_aug,
        "wo_aug": wo_aug,
        "wow1": wow1,
        "w1_aug": w1_aug,
        "w2_ch": np.ascontiguousarray(w2_ch),
        "b2_col": np.ascontiguousarray((b2 + bo).reshape(DEPTH, 1)),
        "coefs": coefs,
        "pcoefs": pcoefs,
        "wqk0s": wqk0s,
        "ident": ident,
        "identb": identb,
    }


def build_module(n_steps=N_STEPS, score_mult=1, ablate="none"):
    import concourse.bass as bass  # noqa: F401
    import concourse.mybir as mybir
    import concourse.tile as tile
    from concourse import bacc

    f32 = mybir.dt.float32
    f32r = mybir.dt.float32r
    bf = mybir.dt.bfloat16

    nc = bacc.Bacc("TRN2", target_bir_lowering=False, debug=False, enable_asserts=False,
                   num_devices=N_CORES)

    # --- DRAM I/O ---
    x_d = nc.dram_tensor("x", [E, S, DEPTH], f32, kind="ExternalInput").ap()
    wqk_d = nc.dram_tensor("wqk", [2, HPL, DEPTH + 1, 128], f32r, kind="ExternalInput").ap()
    wv_d = nc.dram_tensor("wv_aug", [DEPTH + 1, HL * DH], f32r, kind="ExternalInput").ap()
    wo_d = nc.dram_tensor("wo_aug", [DEPTH + 1, DEPTH], bf, kind="ExternalInput").ap()
    wow1_d = nc.dram_tensor("wow1", [DEPTH, DFF], bf, kind="ExternalInput").ap()
    w1_d = nc.dram_tensor("w1_aug", [DEPTH + 1, DFF], bf, kind="ExternalInput").ap()
    w2_d = nc.dram_tensor("w2_ch", [2, 128, DEPTH], bf, kind="ExternalInput").ap()
    b2_d = nc.dram_tensor("b2_col", [DEPTH, 1], f32, kind="ExternalInput").ap()
    coef_d = nc.dram_tensor("coefs", [12, 128, DEPTH], f32r, kind="ExternalInput").ap()
    pcoef_d = nc.dram_tensor("pcoefs", [8, 128, DEPTH], f32r, kind="ExternalInput").ap()
    wqk0s_d = nc.dram_tensor("wqk0s", [6, 2, 128, 128], f32r, kind="ExternalInput").ap()
    id_d = nc.dram_tensor("ident", [128, 128], f32, kind="ExternalInput").ap()
    idb_d = nc.dram_tensor("identb", [128, 128], bf, kind="ExternalInput").ap()
    y_d = nc.dram_tensor("y_out", [E, S, DEPTH], f32, kind="ExternalOutput").ap()

    coef_stage_index = []
    idx = 0
    for pairs in _coef_list():
        entry = []
        for (pi, _, _) in pairs:
            entry.append((pi, idx))
            idx += 1
        coef_stage_index.append(entry)
    pcoef_stage_index = []
    idx = 0
    for pairs in _partial_coef_list():
        entry = []
        for (pi, _, _) in pairs:
            entry.append((pi, idx))
            idx += 1
        pcoef_stage_index.append(entry)

    with tile.TileContext(nc) as tc:
        with (
            tc.tile_pool(name="persist", bufs=1) as pp,
            tc.tile_pool(name="psc", bufs=2, space="PSUM") as psc,
            tc.tile_pool(name="pctx", bufs=2, space="PSUM") as pctx,
            tc.tile_pool(name="psm", bufs=2, space="PSUM") as psm,
            tc.tile_pool(name="ccp", bufs=4, space="DRAM") as ccp,
        ):
            # ---- persistent SBUF (per-element state has a leading E dim) ----
            ident = pp.tile([128, 128], f32)
            identb = pp.tile([128, 128], bf)
            wqk_sb = pp.tile([DEPTH + 1, 2 * HPL, 128], f32r)     # [p, (w hp), m]
            wv_sb = pp.tile([DEPTH + 1, HL * DH], f32r)
            wo_sb = pp.tile([DEPTH + 1, DEPTH], bf)
            wow1_sb = pp.tile([DEPTH, DFF], bf)
            w1_sb = pp.tile([DEPTH + 1, DFF], bf)
            w2_sb = pp.tile([128, 2, DEPTH], bf)
            b2_sb = pp.tile([DEPTH, 1], f32)
            coef_sb = pp.tile([128, 12, DEPTH], f32r)
            pcoef_sb = pp.tile([128, 8, DEPTH], f32r)
            wqk0s_sb = pp.tile([128, 6, 2, 128], f32r)
            partial_sb = pp.tile([DEPTH + 1, E, 2, S], f32r)
            yT = pp.tile([DEPTH + 1, E, S], f32)
            ypT = pp.tile([DEPTH + 1, E, 2, S], f32r)
            kp_sb = pp.tile([128, E, 3, S], f32r)
            qkt_sb = pp.tile([128, E, HPL, 2, S], bf)  # [:, e, hp, 0]=Q, 1=K
            v_sb = pp.tile([128, E, 4, 9 * HL], bf)
            expT = pp.tile([128, E, HL, 2, 1024], bf)
            ctxn = pp.tile([128, E, 4, HL * DH], bf)
            ctxT_sb = pp.tile([DEPTH + 1, E, S], bf)
            zT_sb = pp.tile([DEPTH + 1, E, S], bf)
            hT_sb = pp.tile([128, E, 2, S], bf)
            recipZ = pp.tile([128, E, 4, HL], f32)
            ctxTmy = pp.tile([16, E, 2, S], bf)  # [chunk-part, e, g2, tokens]
            xn_sb = pp.tile([128, E, 4, DEPTH], f32)
            yout_sb = pp.tile([128, E, 4, DEPTH], f32)

            # ---- prologue: weight DMAs + presets ----
            nc.sync.dma_start(ident, id_d)
            nc.sync.dma_start(identb, idb_d)
            nc.sync.dma_start(wqk_sb, wqk_d.rearrange("w h p m -> p (w h) m"))
            nc.sync.dma_start(wv_sb, wv_d)
            nc.sync.dma_start(wo_sb, wo_d)
            nc.sync.dma_start(wow1_sb, wow1_d)
            nc.sync.dma_start(w1_sb, w1_d)
            nc.sync.dma_start(w2_sb, w2_d.rearrange("c p m -> p c m"))
            nc.sync.dma_start(b2_sb, b2_d)
            nc.sync.dma_start(coef_sb, coef_d.rearrange("s p m -> p s m"))
            nc.sync.dma_start(pcoef_sb, pcoef_d.rearrange("s p m -> p s m"))
            nc.sync.dma_start(wqk0s_sb, wqk0s_d.rearrange("s w p m -> p s w m"))
            nc.sync.dma_start(xn_sb, x_d.rearrange("e (c p) d -> p e c d", p=128))

            nc.vector.memset(kp_sb.bitcast(f32), 0.0)
            nc.vector.memset(yT[DEPTH:DEPTH + 1, :, :], 1.0)
            nc.vector.memset(ypT.bitcast(f32)[DEPTH:DEPTH + 1, :, :, :], 1.0)
            nc.vector.memset(partial_sb.bitcast(f32)[DEPTH:DEPTH + 1, :, :, :], 1.0)
            nc.vector.memset(ctxT_sb[DEPTH:DEPTH + 1, :, :], 1.0)
            nc.vector.memset(zT_sb[DEPTH:DEPTH + 1, :, :], 1.0)
            v4 = v_sb.rearrange("p e c (h n) -> p e c h n", h=HL)
            nc.vector.memset(v_sb, 1.0)  # ones column at [..., 8] survives V writes
            if ablate == "noexp":
                nc.vector.memset(expT, 1.0)

            tc.strict_bb_all_engine_barrier()

            # transpose input x -> yT
            for e in range(E):
                yt0 = psm.tile([DEPTH, S], f32, tag="sm")
                for qc in range(4):
                    nc.tensor.transpose(
                        yt0[:, 128 * qc:128 * qc + 128], xn_sb[:, e, qc, :], ident
                    )
                nc.vector.tensor_copy(yT[:DEPTH, e, :], yt0)

            def emit_qk(e, yp_ap, hp):
                t = psc.tile([128, 1024], f32, tag="sc")
                nc.tensor.matmul(t[:, 0:512], wqk_sb[:, hp, :], yp_ap)
                nc.tensor.matmul(t[:, 512:1024], wqk_sb[:, HPL + hp, :], yp_ap)
                nc.vector.tensor_copy(
                    qkt_sb[:, e, hp, :, :],
                    t.rearrange("p (w n) -> p w n", w=2),
                )

            VW = HL * DH  # V output width (local heads)

            def emit_v(e, yp_ap):
                tv = psm.tile([128, 4 * VW], f32, tag="sm")
                for c in range(4):
                    nc.tensor.matmul(
                        tv[:, VW * c:VW * (c + 1)],
                        yp_ap[:, 128 * c:128 * c + 128], wv_sb,
                    )
                nc.vector.tensor_copy(
                    v4[:, e, :, :, :DH],
                    tv.rearrange("p (c h n) -> p c h n", c=4, h=HL),
                )

            def emit_pack(e, hp, kp):
                tiles = []
                for hsel in range(2):
                    t = psc.tile([128, 1024], f32, tag="sc")
                    tiles.append(t)
                    for rep in range(score_mult):
                        for rr in range(2):
                            r = hsel + 2 * rr  # row group; rr selects key chunk
                            kc = 2 * kp + rr
                            for c in range(4):
                                nc.tensor.matmul(
                                    t[32 * c:32 * c + 32, 512 * rr:512 * rr + 512],
                                    qkt_sb[:, e, hp, 1, :][
                                        32 * r:32 * r + 32,
                                        128 * kc + 32 * c:128 * kc + 32 * c + 32,
                                    ],
                                    qkt_sb[:, e, hp, 0, :][32 * r:32 * r + 32, :],
                                    tile_position=(32 * r, 32 * c),
                                )
                import concourse.mybir as mybir
                if ablate != "noexp":
                    for hsel in range(2):
                        h = 2 * hp + hsel
                        nc.scalar.activation(
                            expT[:, e, h, kp, :], tiles[hsel][:, :],
                            mybir.ActivationFunctionType.Exp, scale=SCALE,
                        )

            ctx_state = {}

            def emit_av(st, e, h):
                import concourse.mybir as mybir
                ctx_ps = st["ctx_ps"]
                for qc in range(4):
                    for kc in range(4):
                        nc.tensor.matmul(
                            ctx_ps[:, 9 * (HL * qc + h):9 * (HL * qc + h) + 9],
                            expT[:, e, h, kc // 2,
                                 (kc % 2) * 512 + 128 * qc:(kc % 2) * 512 + 128 * qc + 128],
                            v_sb[:, e, kc, 9 * h:9 * h + 9],
                            start=(kc == 0), stop=(kc == 3),
                        )

            def emit_ctx_chunk(st, e, g2):
                """AV + norm + transpose + pair-AllGather for local heads
                (2g2, 2g2+1).  Chunk 0 is emitted mid-ph1 so its exchange
                hides under the remaining exp train."""
                import concourse.mybir as mybir
                c3 = st["c3"]
                emit_av(st, e, 2 * g2)
                emit_av(st, e, 2 * g2 + 1)
                hs = slice(2 * g2, 2 * g2 + 2)
                tps = psm.tile([16, S], bf, tag="sm")
                for qc in range(4):
                    nc.vector.reciprocal(recipZ[:, e, qc, hs], c3[:, qc, hs, DH])
                    nc.vector.tensor_tensor(
                        ctxn[:, e, qc, 16 * g2:16 * g2 + 16].rearrange(
                            "p (h n) -> p h n", h=2),
                        c3[:, qc, hs, :DH],
                        recipZ[:, e, qc, hs, None].to_broadcast((128, 2, DH)),
                        mybir.AluOpType.mult,
                    )
                    nc.tensor.transpose(
                        tps[:, 128 * qc:128 * qc + 128],
                        ctxn[:, e, qc, 16 * g2:16 * g2 + 16], identb,
                    )
                nc.vector.tensor_copy(ctxTmy[:, e, g2, :], tps)
                cci = ccp.tile([16, S], bf, tag="cci")
                cco = ccp.tile([SPLIT, 16, S], bf, tag="cco")
                nc.sync.dma_start(cci, ctxTmy[:, e, g2, :])
                nc.gpsimd.collective_compute(
                    "AllGather", mybir.AluOpType.bypass, CC_GROUPS,
                    ins=[cci[:]], outs=[cco[:]],
                )
                st["cco%d" % g2] = cco

            def emit_ph1(e, yp_ap, qk0_ready):
                """Scores + exps for the local packs; QKV projections and the
                first ctx-chunk exchange woven in."""
                ctx_ps = pctx.tile([128, 4 * 9 * HL], f32, tag="ctx")
                st = {"ctx_ps": ctx_ps,
                      "c3": ctx_ps.rearrange("p (q h n) -> p q h n", q=4, h=HL)}
                ctx_state[e] = st
                if not qk0_ready:
                    emit_qk(e, yp_ap, 0)
                emit_pack(e, 0, 0)
                emit_qk(e, yp_ap, 1)
                emit_pack(e, 0, 1)
                emit_v(e, yp_ap)
                emit_pack(e, 1, 0)
                emit_ctx_chunk(ctx_state[e], e, 0)
                emit_pack(e, 1, 1)

            def emit_tail(e, yp_ap, k_slot, next_sc):
                """Second ctx chunk + gathered Wo/FFN (WoW1 fold) + k evac +
                next-dyn shortcut."""
                import concourse.mybir as mybir
                st = ctx_state.pop(e)
                attk_ps = pctx.tile([DEPTH, S], f32, tag="ctx")
                emit_ctx_chunk(st, e, 1)

                # next dyn's partial (during the CC round trip)
                mm1_rhs = None
                if next_sc is not None:
                    sidx, mm1_rhs, pair_idx, pemit, pslot = next_sc
                    if pemit is not None:
                        d_p = psm.tile([DEPTH, S], f32, tag="sm")
                        for j, (pi, ci) in enumerate(pemit):
                            nc.tensor.matmul(
                                d_p, pcoef_sb[:, ci, :], kp_sb[:, e, pi, :],
                                start=(j == 0), stop=(j == len(pemit) - 1),
                            )
                        nc.vector.tensor_add(
                            partial_sb[:DEPTH, e, pslot, :], yT[:DEPTH, e, :], d_p)
                        mm1_rhs = partial_sb[:, e, pslot, :]

                # FFN part A: th = W1 @ yp (bf16 copy), no exchange dep
                th_tiles = []
                for ch in range(2):
                    th = psm.tile([128, S], f32, tag="sm")
                    nc.tensor.matmul(th, w1_sb[:, 128 * ch:128 * ch + 128],
                                     zT_sb[:, e, :], start=True, stop=False,
                                     skip_group_check=True)
                    th_tiles.append(th)

                # gather read-back: canonical head-row order [0,1,4,5,2,3,6,7]
                nc.sync.dma_start(ctxT_sb[0:16, e, :], st["cco0"][0])
                nc.gpsimd.dma_start(ctxT_sb[16:32, e, :], st["cco0"][1])
                nc.sync.dma_start(ctxT_sb[32:48, e, :], st["cco1"][0])
                nc.gpsimd.dma_start(ctxT_sb[48:DEPTH, e, :], st["cco1"][1])

                nc.tensor.matmul(attk_ps, wo_sb[0:DEPTH, :], ctxT_sb[0:DEPTH, e, :],
                                 start=True, stop=True, skip_group_check=True)

                yp_f = yp_ap.bitcast(f32)
                ksl = kp_sb[64 * (k_slot % 2):64 * (k_slot % 2) + 64, e,
                            k_slot // 2, :]
                if ablate != "noffn":
                    # FFN part B: th += (Wo@W1) @ ctx ; then relu, W2
                    for ch in range(2):
                        nc.tensor.matmul(th_tiles[ch],
                                         wow1_sb[:, 128 * ch:128 * ch + 128],
                                         ctxT_sb[0:DEPTH, e, :],
                                         start=False, stop=True,
                                         skip_group_check=True)
                        nc.vector.tensor_scalar_max(hT_sb[:, e, ch, :],
                                                    th_tiles[ch], 0.0)
                    for ch in range(2):
                        nc.tensor.matmul(
                            attk_ps, w2_sb[:, ch, :], hT_sb[:, e, ch, :],
                            start=False, stop=(ch == 1), skip_group_check=True,
                        )
                # shortcut Q|K for the next dyn's hp0: two 1-bank psm
                # tiles (NOT the psc rotation, where the allocation would
                # WAR-wait on the other element's last exp).  mm1 (partial
                # part) needs no k and overlaps the k evac below; mm2 adds
                # a*QK(k_last); both are evacuated to SBUF here in the tail.
                qk_next = None
                if next_sc is not None:
                    qkQ = psm.tile([128, S], f32, tag="sm")
                    qkK = psm.tile([128, S], f32, tag="sm")
                    qk_next = (qkQ, qkK)
                    nc.tensor.matmul(qkQ, wqk_sb[:, 0, :], mm1_rhs,
                                     start=True, stop=False, skip_group_check=True)
                    nc.tensor.matmul(qkK, wqk_sb[:, HPL, :], mm1_rhs,
                                     start=True, stop=False, skip_group_check=True)
                nc.vector.tensor_scalar(
                    ksl, attk_ps, b2_sb, DT,
                    mybir.AluOpType.add, mybir.AluOpType.mult,
                )
                if qk_next is not None:
                    sidx = next_sc[0]
                    pair_idx = next_sc[2]
                    nc.tensor.matmul(qkQ, wqk0s_sb[:, sidx, 0, :],
                                     kp_sb[:, e, pair_idx, :],
                                     start=False, stop=True, skip_group_check=True)
                    nc.tensor.matmul(qkK, wqk0s_sb[:, sidx, 1, :],
                                     kp_sb[:, e, pair_idx, :],
                                     start=False, stop=True, skip_group_check=True)
                    nc.vector.tensor_copy(qkt_sb[:, e, 0, 0, :], qkQ)
                    nc.vector.tensor_copy(qkt_sb[:, e, 0, 1, :], qkK)
                    return True
                return None

            def emit_delta(e, stage_pairs):
                d_ps = psm.tile([DEPTH, S], f32, tag="sm")
                n = len(stage_pairs)
                for j, (pi, ci) in enumerate(stage_pairs):
                    nc.tensor.matmul(
                        d_ps, coef_sb[:, ci, :],
                        kp_sb[:, e, pi, :],
                        start=(j == 0), stop=(j == n - 1),
                    )
                return d_ps

            use_sc = (ablate == "none") or (ablate == "noffn")
            qk0_ready = [None] * E
            yp_aps = [None] * E

            def emit_head(e, step, st):
                """Stage-head for element e: build yp, project Q/K (hp1-3),
                V, all score packs + exps."""
                if st == 0:
                    yp_ap = ypT[:, e, 0, :]
                    nc.vector.tensor_copy(yp_ap[:DEPTH, :], yT[:DEPTH, e, :])
                    if use_sc and qk0_ready[e] is None:
                        # very first dyn: direct hp0 QK from y
                        qk0 = psc.tile([128, 1024], f32, tag="sc")
                        nc.tensor.matmul(qk0[:, 0:512], wqk_sb[:, 0, :], yp_ap)
                        nc.tensor.matmul(qk0[:, 512:1024], wqk_sb[:, HPL, :], yp_ap)
                        nc.vector.tensor_copy(
                            qkt_sb[:, e, 0, :, :],
                            qk0.rearrange("p (w n) -> p w n", w=2))
                        qk0_ready[e] = True
                else:
                    d_ps = emit_delta(e, coef_stage_index[st - 1])
                    yp_ap = ypT[:, e, st % 2, :]
                    nc.vector.tensor_add(yp_ap[:DEPTH, :], yT[:DEPTH, e, :], d_ps)
                yp_aps[e] = yp_ap
                nc.vector.tensor_copy(zT_sb[:DEPTH, e, :],
                                      yp_ap.bitcast(f32)[:DEPTH, :])
                emit_ph1(e, yp_ap, use_sc and qk0_ready[e])

            def next_sc_for(step, st):
                if not use_sc:
                    return None
                last = (step == n_steps - 1) and (st == 5)
                if last:
                    return None
                if st < 5:
                    j = st + 1  # next stage 1..5, shortcut idx j-1
                    pemit = (None if j < 2 else pcoef_stage_index[j - 2])
                    return (j - 1, None if j > 1 else None, (j - 1) // 2,
                            pemit, j % 2)
                # next is stage 0 of the next step: partial_f route
                return (5, None, 2, pcoef_stage_index[4], 0)

            # Flat (step, stage) schedule.  Per slot and element:
            #   tail(e, st) ; [yT update if st==5] ; head(e, next slot)
            # so ACT's FIFO alternates A-exps / B-exps with each element's
            # tail overlapped by the other element's exp train.
            seq = [(step, st) for step in range(n_steps) for st in range(6)]
            for e in range(E):
                emit_head(e, *seq[0])
            for i, (step, st) in enumerate(seq):
                for e in range(E):
                    sc = next_sc_for(step, st)
                    if sc is not None and st == 0:
                        # stage-1 shortcut mm1 reads yp directly
                        sc = (sc[0], yp_aps[e], sc[2], sc[3], sc[4])
                    r = emit_tail(e, yp_aps[e], st, sc)
                    if use_sc:
                        qk0_ready[e] = r
                    if st == 5:
                        d_ps = emit_delta(e, coef_stage_index[5])
                        nc.vector.tensor_add(yT[:DEPTH, e, :], yT[:DEPTH, e, :],
                                             d_ps)
                    if i + 1 < len(seq):
                        emit_head(e, *seq[i + 1])

            # ---- epilogue: transpose yT back and store ----
            for e in range(E):
                yo = pctx.tile([128, 288], f32, tag="ctx")
                for qc in range(4):
                    nc.tensor.transpose(
                        yo[:, DEPTH * qc:DEPTH * (qc + 1)],
                        yT[:DEPTH, e, 128 * qc:128 * qc + 128],
                        ident[:DEPTH, :DEPTH],
                    )
                nc.vector.tensor_copy(
                    yout_sb[:, e, :, :], yo[:, :4 * DEPTH].rearrange(
                        "p (c d) -> p c d", c=4)
                )
            nc.sync.dma_start(y_d.rearrange("e (c p) d -> p e c d", p=128), yout_sb)

    nc.compile()
    return nc


def make_core_inputs(x, auxs, c):
    """Core c belongs to pair g = c//2 and has role r = c%2 (head half).
    Pair g computes elements (E*g+i)%B for i<E (pairs past B//E duplicate)."""
    g, r = c // 2, c % 2
    xe = np.stack([x[(E * g + i) % B] for i in range(E)])
    m = {"x": np.ascontiguousarray(xe)}
    m.update(auxs[r])
    return m


def gather_output(results):
    # element k is computed (replicated across the pair) on core 2*(k//E)
    return np.stack(
        [results[2 * (k // E)]["y_out"][k % E] for k in range(B)]
    ).astype(np.float32)


def _run(inputs, **spmd_kwargs):
    x = np.asarray(inputs["x"], np.float32)
    auxs = [prepare_aux(inputs, role=r) for r in range(SPLIT)]

    nc = build_module()

    in_maps = [make_core_inputs(x, auxs, c) for c in range(N_CORES)]

    from concourse.bass_utils import run_bass_kernel_spmd

    try:
        res = run_bass_kernel_spmd(nc, in_maps, core_ids=list(range(N_CORES)),
                                   **spmd_kwargs)
    except Exception:
        # transient axon/nrt flakes (e.g. "mesh desynced") recover on retry
        res = run_bass_kernel_spmd(nc, in_maps, core_ids=list(range(N_CORES)),
                                   **spmd_kwargs)
    return gather_output(res.results), res


def kernel(**inputs):
    return _run(inputs)[0]

